# revision 19
# baseline (speedup 1.0000x reference)
"""DeepseekMoE block (attention + top-2 routed MoE + shared expert) on 8 TRN2
NeuronCores, data-parallel over the batch dimension (B=8 -> one batch per core).

End-to-end latency here is dominated by host<->device transfer through the
tunnel (per-tensor fixed cost + ~75MB/s), so the kernel minimizes both shipped
bytes and tensor count:
  - ALL per-core inputs ride in ONE packed f16 blob (bf16/f32 sections are
    bitcast views into it), so each call moves exactly two device buffers:
    the blob and the donated output buffer.
  - Weights are shipped SHARDED 1/8th per core and reassembled on-device with
    HBM->HBM AllGather collectives (each weight byte crosses the link once
    instead of 8x).  Weights are pre-transposed on host to [K_in, M_out]
    (on-device XBAR DMA-transpose was tried and races -- InstDmaTransposeAnt
    semaphore increments are hardcoded to 16, breaking tile-assigned waits).
  - The attention chain (x, qkv/out_proj weights, scores, ctx) runs in fp16
    (half the bytes of fp32 at ~8x less noise than bf16 -- the router's top-2
    selection is sensitive to noise in x + attn_out). Expert FFNs run in bf16.
    Router logits stay fp32.
  - The output is quantized on-device to int8 with a per-token scale
    (QCLIP=4.2 rms clip; the f32 scale row rides in 4 extra rows of the int8
    output tensor), halving both the output download and the donated
    zero-buffer upload. Host dequantizes.
  - The token dimension is truncated to LP = ceil(max(true_counts)/128)*128;
    padded tokens are masked as attention keys and zeroed at the output, so
    they cannot influence valid outputs.

Layout strategy per core (LP tokens, H=1024 hidden): activations live in
"F-layout" [feature-on-partitions, tokens-on-free]; per-token scalars are
produced as [1, LP] rows and broadcast across partitions with K=1 rank-1
matmuls on the TensorEngine; attention is computed transposed (attT[k, q]) so
the key-padding mask and exp() fold into one scalar-engine activation.
"""

import numpy as np
import ml_dtypes
from contextlib import ExitStack

import concourse.bass as bass
import concourse.mybir as mybir
import concourse.tile as tile
from concourse import bacc
from concourse.bass_utils import run_bass_kernel_spmd
from concourse.masks import make_identity

B, L, H = 8, 1024, 1024
E, I, NH, HD = 8, 256, 4, 256
ISZ = 512
P = 128
KH = H // P      # hidden slabs (8)
EPS = 1e-6
NEG = -30000.0
INV_SQRT_HD = float(1.0 / np.sqrt(HD))
NCORES = 8
NKD = 2 * E + ISZ // P   # down-proj K slabs (20)
QCLIP = 4.2              # int8 clip range in units of per-token rms(Y)

DT = mybir.dt
F32, BF16, F16, I32 = DT.float32, DT.bfloat16, DT.float16, DT.int32
F32R = DT.float32r
Alu = mybir.AluOpType
Act = mybir.ActivationFunctionType
AX = mybir.AxisListType

# transposed ([K_in, M_out]) weight matrix shapes; sharded by rows across cores
RAW_SHAPES = {
    "wattn": (4 * H, H),       # RAW [wq|wk|wv rows (ipw*cnw) | wo rows] f16
    "wgu": (H, 2 * E * I),     # [wgT|wuT] bf16 (host-transposed)
    "wsgu": (H, 2 * ISZ),      # [wsgT|wsuT] bf16 (host-transposed)
    "wd": (E * I + ISZ, H),    # [routed down (EI rows) | shared down] bf16
}

# blob32 [P, 90] f32 column layout: wgt k-slabs | bqk | bop | tc | ogb
B32_WGT, B32_BQK, B32_BOP, B32_TC, B32_OGB, B32_W = 0, 64, 80, 88, 89, 90


def blob16_sections(LP):
    """(name -> (offset, n_elems)) f16-element layout of the packed blob."""
    secs = {}
    off = 0
    items = [("x", H * LP)]
    items += [(k, (r * c) // NCORES) for k, (r, c) in RAW_SHAPES.items()]
    items += [("bv", H), ("ogc", H), ("b32", P * B32_W * 2)]
    for name, n in items:
        secs[name] = (off, n)
        off += n
    return secs, off


def build(NT):
    LP = NT * P
    # token-dim chunks (moving free dim <= 512, one PSUM bank each)
    JT = [(0, LP)] if LP <= 512 else [(0, 512), (512, LP - 512)]
    JH = [(0, 512), (512, 512)]  # hidden-dim chunks (always H=1024)

    nc = bacc.Bacc("TRN2", target_bir_lowering=False, debug=False,
                   num_devices=NCORES, enable_partition_id=False)

    secs, n16 = blob16_sections(LP)
    blob = nc.dram_tensor("blob16", [n16], F16, kind="ExternalInput").ap()

    def sec(name, rows, dt=F16):
        off, n = secs[name]
        ap = blob[off:off + n]
        if dt != F16:
            ap = ap.bitcast(dt)
            n = ap.shape[0]
        return ap.rearrange("(p f) -> p f", p=rows)

    xR = sec("x", LP)                             # raw x [LP, H] f16
    bvr = sec("bv", 1)                            # [1, H] f16
    ogm = sec("ogc", P, BF16)                     # [P, KH] bf16
    b32 = sec("b32", P, F32)                      # [P, 90] f32
    # int8 output [H, LP] + 4 trailing rows carrying the per-token f32
    # dequant scale (rms(Y)*sigmoid*(5/127)) as raw bytes
    oh = nc.dram_tensor("out", [H + 4, LP], DT.int8, kind="ExternalOutput")
    outm = oh.ap()
    out1d = oh.reshape([(H + 4) * LP]).ap()

    RG = [list(range(NCORES))]

    with tile.TileContext(nc) as tc:
        es = {}  # manually closed long-lived pools

        def open_pool(key, **kw):
            st = ExitStack()
            pool = st.enter_context(tc.tile_pool(name=key, **kw))
            es[key] = st
            return pool

        with ExitStack() as top:
            const = top.enter_context(tc.tile_pool(name="const", bufs=1))

            ident = const.tile([P, P], F32, name="ident")
            make_identity(nc, ident)
            ident_h = const.tile([P, P], F16, name="ident_h")
            nc.scalar.copy(ident_h[:], ident[:])
            ones_cb = const.tile([P, 1], BF16, name="ones_cb")
            nc.gpsimd.memset(ones_cb[:], 1.0)
            ones_ch = const.tile([P, 1], F16, name="ones_ch")
            nc.gpsimd.memset(ones_ch[:], 1.0)
            ones_bc_f = const.tile([65, P], F32, name="ones_bc_f")
            nc.gpsimd.memset(ones_bc_f[:], 1.0)
            ones_bc = const.tile([65, P], F32R, name="ones_bc")
            nc.scalar.copy(ones_bc[:], ones_bc_f[:])
            ones_row = ones_bc[0:1, :]
            ones_row_h = const.tile([1, P], F16, name="ones_row_h")
            nc.gpsimd.memset(ones_row_h[:], 1.0)
            eps_col = const.tile([P, 1], F32, name="eps_col")
            nc.gpsimd.memset(eps_col[:], EPS)
            tc_sb = const.tile([P, 1], F32, name="tc_sb")
            nc.sync.dma_start(tc_sb[:], b32[:, B32_TC:B32_TC + 1])

            # key-padding masks: maskc[:, kb] = 0 if (kb*128+p) < tc else NEG
            iog = const.tile([P, NT], I32, name="iog")
            nc.gpsimd.iota(iog[:], pattern=[[P, NT]], base=0, channel_multiplier=1)
            iogf = const.tile([P, NT], F32, name="iogf")
            nc.vector.tensor_copy(iogf[:], iog[:])
            mask01 = const.tile([P, NT], F32, name="mask01")
            nc.vector.tensor_scalar(mask01[:], iogf[:], tc_sb[:], None, op0=Alu.is_ge)
            maskc = const.tile([P, NT], F32, name="maskc")
            nc.scalar.mul(maskc[:], mask01[:], NEG)
            # valid[0, n] = 1 if n < tc else 0
            ior = const.tile([1, LP], I32, name="ior")
            nc.gpsimd.iota(ior[:], pattern=[[1, LP]], base=0, channel_multiplier=0)
            iorf = const.tile([1, LP], F32, name="iorf")
            nc.vector.tensor_copy(iorf[:], ior[:])
            valid = const.tile([1, LP], F32, name="valid")
            nc.vector.tensor_scalar(valid[:], iorf[:], tc_sb[0:1, :], None, op0=Alu.is_lt)

            # ---- weight allgather: blob shard -> bounce -> gathered raw ----
            # gpsimd runs these after the const memset/iota above; weight
            # loads (sync-engine XBAR-transpose DMAs) wait on the matching
            # gather, so the gathers overlap phase A compute.
            dramp = top.enter_context(tc.tile_pool(name="dramw", bufs=1,
                                                   space="DRAM"))
            G = {}
            for name, (rows, cols) in RAW_SHAPES.items():
                dt = F16 if name == "wattn" else BF16
                bnc = dramp.tile([rows // NCORES, cols], dt, name=f"{name}_b")
                G[name] = dramp.tile([rows, cols], dt, name=f"{name}_g")
                nc.gpsimd.dma_start(bnc[:], sec(name, rows // NCORES, dt))
                nc.gpsimd.collective_compute(
                    "AllGather", Alu.bypass, replica_groups=RG,
                    ins=[bnc[:].opt()], outs=[G[name][:].opt()])
            wattn_g, wgu_g, wsgu_g, wd_g = (G["wattn"], G["wgu"],
                                            G["wsgu"], G["wd"])

            bias_p = top.enter_context(tc.tile_pool(name="biasp", bufs=1))
            bqk_sb = bias_p.tile([P, 16], F32, name="bqk")
            nc.sync.dma_start(bqk_sb[:], b32[:, B32_BQK:B32_BQK + 16])
            bvr_sb = bias_p.tile([1, H], F16, name="bvr")
            nc.sync.dma_start(bvr_sb[:], bvr[:, :])
            bop_sb = bias_p.tile([P, KH], F32, name="bop")
            nc.sync.dma_start(bop_sb[:], b32[:, B32_BOP:B32_BOP + KH])

            # ---------------- phase A: rms0 + nx ----------------
            # x arrives raw [LP, H]; transpose on-device via PE (stat.T @ I)
            # and keep X resident through phase D (saves the reload too)
            wop = open_pool("wo", bufs=1, side="right")
            wo_sb = [wop.tile([P, H], F16, name=f"wo{k}") for k in range(KH)]
            xres = open_pool("xres", bufs=1, side="right")
            X = [xres.tile([P, LP], F16, name=f"x{k}") for k in range(KH)]
            nxp = open_pool("nx", bufs=1, side="right")
            NX = [nxp.tile([P, LP], F16, name=f"nx{k}") for k in range(KH)]
            with ExitStack() as ph:
                with ExitStack() as tph:
                    xp = tph.enter_context(tc.tile_pool(name="xa", bufs=1))
                    ptp2 = tph.enter_context(tc.tile_pool(name="pstp", bufs=2,
                                                          space="PSUM"))
                    xr = []
                    for tb in range(NT):
                        t = xp.tile([P, H], F16, name=f"xr{tb}")
                        nc.sync.dma_start(t[:], xR[tb * P:(tb + 1) * P, :])
                        xr.append(t)
                    for k in range(KH):
                        for tb in range(NT):
                            ps = ptp2.tile([P, P], F32, tag="tp", name="tp")
                            nc.tensor.matmul(ps[:], xr[tb][:, k * P:(k + 1) * P],
                                             ident_h[:], start=True, stop=True)
                            nc.scalar.copy(X[k][:, tb * P:(tb + 1) * P], ps[:])
                sq = ph.enter_context(tc.tile_pool(name="sq0", bufs=KH))
                pp = ph.enter_context(tc.tile_pool(name="ps0", bufs=2, space="PSUM"))
                pb = ph.enter_context(tc.tile_pool(name="ps0b", bufs=2, space="PSUM"))
                bc = ph.enter_context(tc.tile_pool(name="bc0", bufs=1))
                xsq = []
                for k in range(KH):
                    t = sq.tile([P, LP], BF16, tag="xsq", name="xsq")
                    nc.scalar.activation(t[:], X[k][:], Act.Square)
                    xsq.append(t)
                r0row = bc.tile([1, LP], F32, name="r0row")
                sroot = bc.tile([1, LP], F32, name="sroot0")
                for jo, jw in JT:
                    ps = pp.tile([1, 512], F32, tag="ss", name="ss")
                    for k in range(KH):
                        nc.tensor.matmul(ps[:, :jw], ones_cb[:], xsq[k][:, jo:jo + jw],
                                         start=(k == 0), stop=(k == KH - 1))
                    nc.scalar.activation(sroot[0:1, jo:jo + jw], ps[:, :jw],
                                         Act.Sqrt, bias=eps_col[0:1, :], scale=1.0 / H)
                    nc.vector.reciprocal(r0row[0:1, jo:jo + jw],
                                         sroot[0:1, jo:jo + jw])
                r0row_r = bc.tile([1, LP], F32R, name="r0row_r")
                nc.scalar.copy(r0row_r[:], r0row[:])
                r0bc = bc.tile([P, LP], F32, name="r0bc")
                for jo, jw in JT:
                    psb = pb.tile([P, 512], F32, tag="bc", name="bc")
                    nc.tensor.matmul(psb[:, :jw], ones_row[:],
                                     r0row_r[0:1, jo:jo + jw],
                                     start=True, stop=True)
                    nc.scalar.copy(r0bc[:, jo:jo + jw], psb[:, :jw])
                for k in range(KH):
                    nc.vector.tensor_mul(NX[k][:], X[k][:], r0bc[:])

            # ---------------- phase B: QKV ----------------
            qkvp = open_pool("qkv", bufs=1)
            Q = [qkvp.tile([P, LP], F16, name=f"q{i}") for i in range(KH)]
            K = [qkvp.tile([P, LP], F16, name=f"k{i}") for i in range(KH)]
            V = [qkvp.tile([P, H], F16, name=f"v{i}") for i in range(NT)]

            with ExitStack() as ph:
                wp = ph.enter_context(tc.tile_pool(name="wqkv", bufs=1))
                wqk_sb = [wp.tile([P, 2 * H], F16, name=f"wqk_{k}")
                          for k in range(KH)]
                wv_sb = [wp.tile([P, H], F16, name=f"wv{k}") for k in range(KH)]
                with ExitStack() as tph:
                    wsl = tph.enter_context(tc.tile_pool(name="wslab", bufs=3))
                    ptp3 = tph.enter_context(tc.tile_pool(name="pstp2", bufs=2,
                                                          space="PSUM"))
                    for ms in range(32):
                        rs = wsl.tile([P, H], F16, tag="rs", name="rs")
                        nc.sync.dma_start(rs[:], wattn_g[ms * P:(ms + 1) * P, :])
                        for k in range(KH):
                            ps = ptp3.tile([P, P], F32, tag="tp2", name="tp2")
                            nc.tensor.matmul(ps[:], rs[:, k * P:(k + 1) * P],
                                             ident_h[:], start=True, stop=True)
                            if ms < 16:
                                dst, co = wqk_sb[k], ms * P
                            elif ms < 24:
                                dst, co = wv_sb[k], (ms - 16) * P
                            else:
                                dst, co = wo_sb[k], (ms - 24) * P
                            nc.scalar.copy(dst[:, co:co + P], ps[:])
                pp = ph.enter_context(tc.tile_pool(name="psqk", bufs=4, space="PSUM"))
                for fb in range(16):
                    dst = Q[fb] if fb < KH else K[fb - KH]
                    pts = [pp.tile([P, 512], F32, tag="qk", name="qk") for _ in JT]
                    for k in range(KH):
                        for j, (jo, jw) in enumerate(JT):
                            nc.tensor.matmul(
                                pts[j][:, :jw],
                                wqk_sb[k][:, fb * P:(fb + 1) * P],
                                NX[k][:, jo:jo + jw],
                                start=(k == 0), stop=(k == KH - 1))
                    for j, (jo, jw) in enumerate(JT):
                        nc.scalar.activation(dst[:, jo:jo + jw], pts[j][:, :jw],
                                             Act.Identity, bias=bqk_sb[:, fb:fb + 1])
                for tb in range(NT):
                    pts = [pp.tile([P, 512], F32, tag="v", name="v") for _ in JH]
                    for k in range(KH):
                        for j, (jo, jw) in enumerate(JH):
                            nc.tensor.matmul(
                                pts[j][:, :jw],
                                NX[k][:, tb * P:(tb + 1) * P],
                                wv_sb[k][:, jo:jo + jw],
                                start=(k == 0), stop=False)
                    for j, (jo, jw) in enumerate(JH):
                        # homogeneous bias row: out += 1 * bv
                        nc.tensor.matmul(pts[j][:, :jw], ones_row_h[:],
                                         bvr_sb[0:1, jo:jo + jw],
                                         start=False, stop=True)
                        nc.vector.tensor_copy(V[tb][:, jo:jo + jw], pts[j][:, :jw])
            es["nx"].close()

            # ---------------- phase C: attention ----------------
            ctxp = open_pool("ctx", bufs=1, side="right")
            CTX = [ctxp.tile([P, LP], F16, name=f"ctx{i}") for i in range(KH)]
            with ExitStack() as ph:
                ptp = ph.enter_context(tc.tile_pool(name="pt", bufs=NT + 2))
                zp = ph.enter_context(tc.tile_pool(name="zrow", bufs=2))
                zbp = ph.enter_context(tc.tile_pool(name="zbc", bufs=2))
                pa = ph.enter_context(tc.tile_pool(name="psatt", bufs=4, space="PSUM"))
                pz = ph.enter_context(tc.tile_pool(name="psz", bufs=1, space="PSUM"))
                pc = ph.enter_context(tc.tile_pool(name="psctx", bufs=2, space="PSUM"))
                pbb = ph.enter_context(tc.tile_pool(name="psbcz", bufs=1, space="PSUM"))
                for h in range(NH):
                    pts = []
                    for kb in range(NT):
                        pt_t = ptp.tile([P, LP], F16, tag="pt", name="pt")
                        pa_t = [pa.tile([P, 512], F32, tag="att", name="att")
                                for _ in JT]
                        for t in range(2):
                            for j, (jo, jw) in enumerate(JT):
                                nc.tensor.matmul(
                                    pa_t[j][:, :jw],
                                    K[2 * h + t][:, kb * P:(kb + 1) * P],
                                    Q[2 * h + t][:, jo:jo + jw],
                                    start=(t == 0), stop=(t == 1))
                        for j, (jo, jw) in enumerate(JT):
                            nc.scalar.activation(pt_t[:, jo:jo + jw],
                                                 pa_t[j][:, :jw],
                                                 Act.Exp, bias=maskc[:, kb:kb + 1],
                                                 scale=INV_SQRT_HD)
                        pts.append(pt_t)
                    zrow = zp.tile([1, LP], F32, tag="z", name="z")
                    for jo, jw in JT:
                        pz_t = pz.tile([1, 512], F32, tag="z", name="zps")
                        for kb in range(NT):
                            nc.tensor.matmul(pz_t[:, :jw], ones_ch[:],
                                             pts[kb][:, jo:jo + jw],
                                             start=(kb == 0), stop=(kb == NT - 1))
                        nc.vector.reciprocal(zrow[0:1, jo:jo + jw], pz_t[:, :jw])
                    zrow_r = zp.tile([1, LP], F32R, tag="zr", name="zr")
                    nc.scalar.copy(zrow_r[:], zrow[:])
                    zbc = zbp.tile([P, LP], F32, tag="zbc", name="zbc")
                    for jo, jw in JT:
                        pb_t = pbb.tile([P, 512], F32, tag="bcz", name="bcz")
                        nc.tensor.matmul(pb_t[:, :jw], ones_row[:],
                                         zrow_r[0:1, jo:jo + jw],
                                         start=True, stop=True)
                        nc.scalar.copy(zbc[:, jo:jo + jw], pb_t[:, :jw])
                    for db in range(2):
                        pc_t = [pc.tile([P, 512], F32, tag="ctx", name="ctx")
                                for _ in JT]
                        for kb in range(NT):
                            for j, (jo, jw) in enumerate(JT):
                                nc.tensor.matmul(
                                    pc_t[j][:, :jw],
                                    V[kb][:, h * HD + db * P: h * HD + (db + 1) * P],
                                    pts[kb][:, jo:jo + jw],
                                    start=(kb == 0), stop=(kb == NT - 1))
                        for j, (jo, jw) in enumerate(JT):
                            nc.vector.tensor_mul(
                                CTX[2 * h + db][:, jo:jo + jw],
                                pc_t[j][:, :jw], zbc[:, jo:jo + jw])
            es["qkv"].close()

            # ---------------- phase D: out_proj + residual ----------------
            x1p = open_pool("x1", bufs=1)
            X1 = [x1p.tile([P, LP], F32, name=f"x1_{i}") for i in range(KH)]
            with ExitStack() as ph:
                pp = ph.enter_context(tc.tile_pool(name="pso", bufs=4, space="PSUM"))
                for fb in range(KH):
                    pts = [pp.tile([P, 512], F32, tag="o", name="o") for _ in JT]
                    for k in range(KH):
                        for j, (jo, jw) in enumerate(JT):
                            nc.tensor.matmul(
                                pts[j][:, :jw],
                                wo_sb[k][:, fb * P:(fb + 1) * P],
                                CTX[k][:, jo:jo + jw],
                                start=(k == 0), stop=(k == KH - 1))
                    for j, (jo, jw) in enumerate(JT):
                        nc.vector.scalar_tensor_tensor(
                            X1[fb][:, jo:jo + jw],
                            pts[j][:, :jw], bop_sb[:, fb:fb + 1],
                            X[fb][:, jo:jo + jw],
                            op0=Alu.add, op1=Alu.add)
            es["ctx"].close()
            es["xres"].close()
            es["wo"].close()

            # shared-expert weights prefetch (DMA overlaps rms1/gating)
            wexp = open_pool("wexp", bufs=1, side="right")
            wsg_sb, wsu_sb = [], []
            for k in range(KH):
                t = wexp.tile([P, ISZ], BF16, name=f"wsg{k}")
                nc.sync.dma_start(t[:], wsgu_g[k * P:(k + 1) * P, 0:ISZ])
                wsg_sb.append(t)
                t = wexp.tile([P, ISZ], BF16, name=f"wsu{k}")
                nc.sync.dma_start(t[:], wsgu_g[k * P:(k + 1) * P, ISZ:2 * ISZ])
                wsu_sb.append(t)

            # ---------------- phase E: rms1 + xhat + r_cols ----------------
            xhp = open_pool("xhat", bufs=1, side="right")
            XH = [xhp.tile([P, LP], BF16, name=f"xh{k}") for k in range(KH)]
            r_cols = xhp.tile([P, NT], F32, name="r_cols")
            with ExitStack() as ph:
                sq = ph.enter_context(tc.tile_pool(name="sq1", bufs=KH))
                pp = ph.enter_context(tc.tile_pool(name="ps1", bufs=2, space="PSUM"))
                pb = ph.enter_context(tc.tile_pool(name="ps1b", bufs=2, space="PSUM"))
                ptr = ph.enter_context(tc.tile_pool(name="ps1t", bufs=1, space="PSUM"))
                bc = ph.enter_context(tc.tile_pool(name="bc1", bufs=1))
                xsq = []
                for k in range(KH):
                    t = sq.tile([P, LP], BF16, tag="x1sq", name="x1sq")
                    nc.scalar.activation(t[:], X1[k][:], Act.Square)
                    xsq.append(t)
                rrow = bc.tile([1, LP], F32, name="rrow")
                sroot = bc.tile([1, LP], F32, name="sroot1")
                for jo, jw in JT:
                    ps = pp.tile([1, 512], F32, tag="ss", name="ss1")
                    for k in range(KH):
                        nc.tensor.matmul(ps[:, :jw], ones_cb[:], xsq[k][:, jo:jo + jw],
                                         start=(k == 0), stop=(k == KH - 1))
                    nc.scalar.activation(sroot[0:1, jo:jo + jw], ps[:, :jw],
                                         Act.Sqrt, bias=eps_col[0:1, :], scale=1.0 / H)
                    nc.vector.reciprocal(rrow[0:1, jo:jo + jw],
                                         sroot[0:1, jo:jo + jw])
                rrow_r = bc.tile([1, LP], F32R, name="rrow_r")
                nc.scalar.copy(rrow_r[:], rrow[:])
                rbc = bc.tile([P, LP], F32, name="rbc")
                for jo, jw in JT:
                    psb = pb.tile([P, 512], F32, tag="bc", name="bc1")
                    nc.tensor.matmul(psb[:, :jw], ones_row[:],
                                     rrow_r[0:1, jo:jo + jw],
                                     start=True, stop=True)
                    nc.scalar.copy(rbc[:, jo:jo + jw], psb[:, :jw])
                for k in range(KH):
                    nc.vector.tensor_mul(XH[k][:], X1[k][:], rbc[:])
                # r as per-token columns [128, NT] via tiny transposes
                ptt = ptr.tile([P, NT], F32, tag="rt", name="rt")
                for tb in range(NT):
                    nc.tensor.transpose(ptt[:, tb:tb + 1],
                                        rrow[0:1, tb * P:(tb + 1) * P],
                                        ident[0:1, 0:1])
                nc.scalar.copy(r_cols[:], ptt[:])

            # ---------------- phase F: router gating ----------------
            wbcp = open_pool("wbc", bufs=1, side="right")
            WBC = [wbcp.tile([P, LP], BF16, name=f"wbc{e}") for e in range(E)]
            wrows = wbcp.tile([E, LP], F32R, name="wrows")
            # broadcast-source rows live at base partitions 0/32/64 (matmul rule)
            wrow_t = [wbcp.tile([65, LP], F32R, name=f"wrt{i}") for i in range(3)]
            wrow_e = [wrow_t[e // 3][32 * (e % 3):32 * (e % 3) + 1, :] for e in range(E)]
            with ExitStack() as ph:
                wp = ph.enter_context(tc.tile_pool(name="wgate", bufs=1))
                gp = ph.enter_context(tc.tile_pool(name="gating", bufs=4))
                pg = ph.enter_context(tc.tile_pool(name="psg", bufs=4, space="PSUM"))
                pt_ = ph.enter_context(tc.tile_pool(name="psgt", bufs=2, space="PSUM"))
                pwb = ph.enter_context(tc.tile_pool(name="pswb", bufs=2, space="PSUM"))
                wgt_sb = []
                for k in range(KH):
                    t = wp.tile([P, E], F32, name=f"wgt{k}")
                    nc.sync.dma_start(
                        t[:], b32[:, B32_WGT + k * KH:B32_WGT + (k + 1) * KH])
                    wgt_sb.append(t)
                for tb in range(NT):
                    pg_t = pg.tile([P, E], F32, tag="g", name="g")
                    for k in range(KH):
                        nc.tensor.matmul(pg_t[:], X1[k][:, tb * P:(tb + 1) * P], wgt_sb[k][:],
                                         start=(k == 0), stop=(k == KH - 1))
                    s_t = gp.tile([P, E], F32, tag="s", name="s")
                    nc.scalar.activation(s_t[:], pg_t[:], Act.Exp,
                                         scale=r_cols[:, tb:tb + 1])
                    m1 = gp.tile([P, 1], F32, tag="m1", name="m1")
                    nc.vector.reduce_max(m1[:], s_t[:], axis=AX.X)
                    ml = gp.tile([P, E], F32, tag="ml", name="ml")
                    nc.vector.tensor_scalar(ml[:], s_t[:], m1[:], None, op0=Alu.is_lt)
                    s2 = gp.tile([P, E], F32, tag="s2", name="s2")
                    nc.vector.tensor_mul(s2[:], s_t[:], ml[:])
                    m2 = gp.tile([P, 1], F32, tag="m2", name="m2")
                    nc.vector.reduce_max(m2[:], s2[:], axis=AX.X)
                    keep = gp.tile([P, E], F32, tag="keep", name="keep")
                    nc.vector.tensor_scalar(keep[:], s_t[:], m2[:], None, op0=Alu.is_ge)
                    ssum = gp.tile([P, 1], F32, tag="ssum", name="ssum")
                    nc.vector.tensor_add(ssum[:], m1[:], m2[:])
                    srec = gp.tile([P, 1], F32, tag="srec", name="srec")
                    nc.vector.reciprocal(srec[:], ssum[:])
                    wt = gp.tile([P, E], F32, tag="wt", name="wt")
                    nc.vector.scalar_tensor_tensor(wt[:], s_t[:], srec[:], keep[:],
                                                   op0=Alu.mult, op1=Alu.mult)
                    pt_t = pt_.tile([E, P], F32, tag="wtT", name="wtT")
                    nc.tensor.transpose(pt_t[:], wt[:], ident[:])
                    nc.scalar.copy(wrows[:, tb * P:(tb + 1) * P], pt_t[:])
                for e in range(E):
                    nc.sync.dma_start(wrow_e[e][:], wrows[e:e + 1, :])
                for e in range(E):
                    for jo, jw in JT:
                        pw_t = pwb.tile([P, 512], F32, tag="wbc", name="wbcp")
                        base = 32 * (e % 3)
                        nc.tensor.matmul(pw_t[:, :jw], ones_bc[base:base + 1, :],
                                         wrow_e[e][0:1, jo:jo + jw],
                                         start=True, stop=True)
                        nc.scalar.copy(WBC[e][:, jo:jo + jw], pw_t[:, :jw])
            es["x1"].close()

            # ---------------- phase G: routed expert gate/up ----------------
            ap_ = open_pool("acts", bufs=1)
            A = [ap_.tile([P, LP], BF16, name=f"a{i}") for i in range(2 * E)]
            ASH = [ap_.tile([P, LP], BF16, name=f"ash{i}") for i in range(ISZ // P)]
            with ExitStack() as ph:
                tmp = ph.enter_context(tc.tile_pool(name="tmpgu", bufs=2))
                wst = ph.enter_context(tc.tile_pool(name="wgus", bufs=24))
                pp = ph.enter_context(tc.tile_pool(name="psgu", bufs=8, space="PSUM"))
                for fb in range(2 * E):
                    e = fb // 2
                    wgf = []
                    for k in range(KH):
                        t = wst.tile([P, P], BF16, tag="wgs", name="wgs")
                        nc.sync.dma_start(t[:], wgu_g[k * P:(k + 1) * P,
                                                      fb * P:(fb + 1) * P])
                        wgf.append(t)
                    wuf = []
                    for k in range(KH):
                        t = wst.tile([P, P], BF16, tag="wus", name="wus")
                        nc.sync.dma_start(t[:], wgu_g[k * P:(k + 1) * P,
                                                      E * I + fb * P:E * I + (fb + 1) * P])
                        wuf.append(t)
                    pg_ = [pp.tile([P, 512], F32, tag="gu", name="gu") for _ in JT]
                    for k in range(KH):
                        for j, (jo, jw) in enumerate(JT):
                            nc.tensor.matmul(pg_[j][:, :jw], wgf[k][:],
                                             XH[k][:, jo:jo + jw],
                                             start=(k == 0), stop=(k == KH - 1))
                    sgm = tmp.tile([P, LP], BF16, tag="sgm", name="sgm")
                    for j, (jo, jw) in enumerate(JT):
                        nc.scalar.activation(sgm[:, jo:jo + jw], pg_[j][:, :jw],
                                             Act.Sigmoid)
                    sg = tmp.tile([P, LP], BF16, tag="sg", name="sg")
                    for j, (jo, jw) in enumerate(JT):
                        nc.vector.tensor_mul(sg[:, jo:jo + jw], pg_[j][:, :jw],
                                             sgm[:, jo:jo + jw])
                    pu_ = [pp.tile([P, 512], F32, tag="gu", name="gu") for _ in JT]
                    for k in range(KH):
                        for j, (jo, jw) in enumerate(JT):
                            nc.tensor.matmul(pu_[j][:, :jw], wuf[k][:],
                                             XH[k][:, jo:jo + jw],
                                             start=(k == 0), stop=(k == KH - 1))
                    ta = tmp.tile([P, LP], BF16, tag="ta", name="ta")
                    for j, (jo, jw) in enumerate(JT):
                        nc.vector.tensor_mul(ta[:, jo:jo + jw], pu_[j][:, :jw],
                                             sg[:, jo:jo + jw])
                    nc.vector.tensor_mul(A[fb][:], ta[:], WBC[e][:])
            es["wbc"].close()

            # down-proj weights prefetch (DMA overlaps shared expert phase)
            wdp = open_pool("wd", bufs=1)
            wd_sb = []
            for k in range(NKD):
                t = wdp.tile([P, H], BF16, name=f"wd{k}")
                nc.sync.dma_start(t[:], wd_g[k * P:(k + 1) * P, :])
                wd_sb.append(t)

            # ---------------- phase H: shared expert gate/up ----------------
            with ExitStack() as ph:
                tmp = ph.enter_context(tc.tile_pool(name="tmpsgu", bufs=2))
                pp = ph.enter_context(tc.tile_pool(name="pssgu", bufs=8, space="PSUM"))
                for fb in range(ISZ // P):
                    pg_ = [pp.tile([P, 512], F32, tag="sgu", name="sgu") for _ in JT]
                    for k in range(KH):
                        for j, (jo, jw) in enumerate(JT):
                            nc.tensor.matmul(pg_[j][:, :jw],
                                             wsg_sb[k][:, fb * P:(fb + 1) * P],
                                             XH[k][:, jo:jo + jw],
                                             start=(k == 0), stop=(k == KH - 1))
                    sgm = tmp.tile([P, LP], BF16, tag="ssgm", name="ssgm")
                    for j, (jo, jw) in enumerate(JT):
                        nc.scalar.activation(sgm[:, jo:jo + jw], pg_[j][:, :jw],
                                             Act.Sigmoid)
                    sg = tmp.tile([P, LP], BF16, tag="ssg", name="ssg")
                    for j, (jo, jw) in enumerate(JT):
                        nc.vector.tensor_mul(sg[:, jo:jo + jw], pg_[j][:, :jw],
                                             sgm[:, jo:jo + jw])
                    pu_ = [pp.tile([P, 512], F32, tag="sgu", name="sgu") for _ in JT]
                    for k in range(KH):
                        for j, (jo, jw) in enumerate(JT):
                            nc.tensor.matmul(pu_[j][:, :jw],
                                             wsu_sb[k][:, fb * P:(fb + 1) * P],
                                             XH[k][:, jo:jo + jw],
                                             start=(k == 0), stop=(k == KH - 1))
                    for j, (jo, jw) in enumerate(JT):
                        nc.vector.tensor_mul(ASH[fb][:, jo:jo + jw], pu_[j][:, :jw],
                                             sg[:, jo:jo + jw])
            es["xhat"].close()
            es["wexp"].close()

            # ------------- phase I: down proj (routed + shared fused) -------------
            yp = open_pool("y", bufs=1, side="right")
            Y = [yp.tile([P, LP], F32, name=f"y{i}") for i in range(KH)]
            YB = [yp.tile([P, LP], BF16, name=f"yb{i}") for i in range(KH)]
            AALL = A + ASH
            with ExitStack() as ph:
                pp = ph.enter_context(tc.tile_pool(name="psd", bufs=6, space="PSUM"))
                for hb in range(KH):
                    pts = [pp.tile([P, 512], F32, tag="y", name="yps") for _ in JT]
                    for k in range(NKD):
                        for j, (jo, jw) in enumerate(JT):
                            nc.tensor.matmul(pts[j][:, :jw],
                                             wd_sb[k][:, hb * P:(hb + 1) * P],
                                             AALL[k][:, jo:jo + jw],
                                             start=(k == 0), stop=(k == NKD - 1))
                    for j, (jo, jw) in enumerate(JT):
                        nc.scalar.copy(Y[hb][:, jo:jo + jw], pts[j][:, :jw])
                        nc.vector.tensor_copy(YB[hb][:, jo:jo + jw], pts[j][:, :jw])
            es["wd"].close()
            es["acts"].close()

            # ------- phase J: output gate + final mask + int8 quantization -------
            with ExitStack() as ph:
                wp = ph.enter_context(tc.tile_pool(name="wog", bufs=1))
                fr = ph.enter_context(tc.tile_pool(name="final", bufs=1))
                sqy = ph.enter_context(tc.tile_pool(name="sqy", bufs=KH))
                op_ = ph.enter_context(tc.tile_pool(name="outp", bufs=3))
                pg = ph.enter_context(tc.tile_pool(name="psog", bufs=2, space="PSUM"))
                pq = ph.enter_context(tc.tile_pool(name="psq", bufs=2, space="PSUM"))
                pbf = ph.enter_context(tc.tile_pool(name="psfin", bufs=1, space="PSUM"))
                ogc_sb = wp.tile([P, KH], BF16, name="ogc")
                nc.sync.dma_start(ogc_sb[:], ogm[:, :])
                ogb_sb = wp.tile([1, 1], F32, name="ogb")
                nc.sync.dma_start(ogb_sb[:], b32[0:1, B32_OGB:B32_OGB + 1])
                sigrow = fr.tile([1, LP], F32, name="sigrow")
                for jo, jw in JT:
                    pg_t = pg.tile([1, 512], F32, tag="og", name="og")
                    for k in range(KH):
                        nc.tensor.matmul(pg_t[:, :jw], ogc_sb[:, k:k + 1],
                                         YB[k][:, jo:jo + jw],
                                         start=(k == 0), stop=(k == KH - 1))
                    nc.scalar.activation(sigrow[0:1, jo:jo + jw], pg_t[:, :jw],
                                         Act.Sigmoid, bias=ogb_sb[0:1, :])
                # per-token rms(Y) for the int8 scale
                ysq = []
                for k in range(KH):
                    t = sqy.tile([P, LP], BF16, tag="ysq", name="ysq")
                    nc.scalar.activation(t[:], YB[k][:], Act.Square)
                    ysq.append(t)
                rmsrow = fr.tile([1, LP], F32, name="rmsrow")
                for jo, jw in JT:
                    ps = pq.tile([1, 512], F32, tag="yss", name="yss")
                    for k in range(KH):
                        nc.tensor.matmul(ps[:, :jw], ones_cb[:], ysq[k][:, jo:jo + jw],
                                         start=(k == 0), stop=(k == KH - 1))
                    nc.scalar.activation(rmsrow[0:1, jo:jo + jw], ps[:, :jw],
                                         Act.Sqrt, bias=eps_col[0:1, :], scale=1.0 / H)
                rrec = fr.tile([1, LP], F32, name="rrec")
                nc.vector.reciprocal(rrec[:], rmsrow[:])
                # shipped dequant scale: rms * sig * (5/127)
                invq = fr.tile([1, LP], F32, name="invq")
                nc.vector.tensor_mul(invq[:], rmsrow[:], sigrow[:])
                nc.scalar.mul(invq[:], invq[:], QCLIP / 127.0)
                # quantizer broadcast: valid * (127/5) / rms  (sigmoid cancels)
                svq = fr.tile([1, LP], F32, name="svq")
                nc.vector.tensor_mul(svq[:], rrec[:], valid[:])
                svrow = fr.tile([1, LP], F32R, name="svrow")
                nc.scalar.mul(svrow[:], svq[:], 127.0 / QCLIP)
                svb = fr.tile([P, LP], F32, name="svb")
                for jo, jw in JT:
                    pb_t = pbf.tile([P, 512], F32, tag="fin", name="fin")
                    nc.tensor.matmul(pb_t[:, :jw], ones_row[:],
                                     svrow[0:1, jo:jo + jw],
                                     start=True, stop=True)
                    nc.scalar.copy(svb[:, jo:jo + jw], pb_t[:, :jw])
                for hb in range(KH):
                    yt = op_.tile([P, LP], F32, tag="yt", name="yt")
                    nc.vector.tensor_mul(yt[:], Y[hb][:], svb[:])
                    ot = op_.tile([P, LP], DT.int8, tag="ot", name="ot")
                    nc.vector.tensor_scalar(ot[:], yt[:], 127.0, -127.0,
                                            op0=Alu.min, op1=Alu.max)
                    nc.sync.dma_start(outm[hb * P:(hb + 1) * P, :], ot[:])
                inv_i8 = invq[:].bitcast(DT.int8)
                nc.sync.dma_start(out1d[H * LP:(H + 4) * LP], inv_i8)
            es["y"].close()

    nc.compile()
    return nc


_CACHE = {}


def _get_program(NT):
    if NT not in _CACHE:
        _CACHE[NT] = build(NT)
    return _CACHE[NT]


def _prep_inputs(inputs, NT):
    f32 = np.float32
    f16 = np.float16
    bf = ml_dtypes.bfloat16
    LP = NT * P
    g = lambda k: np.asarray(inputs[k]).astype(f32, copy=False)

    hs = g("hidden_states")
    tcs = np.asarray(inputs["true_counts"]).astype(np.int64).reshape(B)
    cnw, gnw, snw = g("context_norm_w"), g("gate_norm_w"), g("shared_norm_w")
    ipw, ipb = g("in_proj_w"), g("in_proj_b")
    opw, opb = g("out_proj_w"), g("out_proj_b")
    gw = g("gate_w")
    enw = g("expert_norm_w")
    egw, euw, edw = g("expert_gate_w"), g("expert_up_w"), g("expert_down_w")
    sgw, suw, sdw = g("shared_gate_w"), g("shared_up_w"), g("shared_down_w")
    ogw, ogb_ = g("out_gate_w"), g("out_gate_b")

    # transposed weight blocks (host-side transposes; device loads are plain).
    # The rmsnorm scale vectors are folded into the weights; skip the
    # multiply when they are all-ones (the common case).
    def fold(wm, nwv, axis=1):
        if np.all(nwv == 1.0):
            return wm
        return wm * (nwv[None, :] if axis == 1 else nwv[:, None])

    raw = {}
    raw["wattn"] = w = np.empty(RAW_SHAPES["wattn"], f16)
    w[:3 * H] = fold(ipw, cnw)
    w[3 * H:] = opw
    egw2 = egw.reshape(E * I, H)
    euw2 = euw.reshape(E * I, H)
    if not np.all(enw == 1.0):
        egw2 = (egw * enw[:, None, :]).reshape(E * I, H)
        euw2 = (euw * enw[:, None, :]).reshape(E * I, H)
    raw["wgu"] = w = np.empty(RAW_SHAPES["wgu"], bf)
    w[:, :E * I] = egw2.T
    w[:, E * I:] = euw2.T
    raw["wsgu"] = w = np.empty(RAW_SHAPES["wsgu"], bf)
    w[:, :ISZ] = fold(sgw, snw).T
    w[:, ISZ:] = fold(suw, snw).T
    raw["wd"] = w = np.empty(RAW_SHAPES["wd"], bf)
    w[:E * I] = edw.transpose(0, 2, 1).reshape(E * I, H)
    w[E * I:] = sdw.T
    bv16 = ipb[2 * H:].astype(f16)
    ogc16 = np.ascontiguousarray(ogw.reshape(KH, P).T.astype(bf)).view(f16)

    # blob32 [P, 90] f32: wgt k-slabs | bqk | bop | tc | ogb
    b32 = np.empty((P, B32_W), f32)
    b32[:, B32_WGT:B32_WGT + 64] = (
        fold(gw, gnw).T.reshape(KH, P, E).transpose(1, 0, 2).reshape(P, 64))
    b32[:, B32_BQK:B32_BQK + 16] = ipb[:2 * H].reshape(16, P).T
    b32[:, B32_BOP:B32_BOP + KH] = opb.reshape(KH, P).T
    b32[:, B32_OGB] = float(ogb_.reshape(-1)[0])

    secs, n16 = blob16_sections(LP)

    def put(blob, name, arr):
        off, n = secs[name]
        blob[off:off + n] = arr.reshape(-1).view(f16)

    in_maps = []
    for b in range(B):
        blob = np.empty(n16, f16)
        o, n = secs["x"]
        blob[o:o + n].reshape(LP, H)[...] = hs[b, :LP]
        for name, (rows, cols) in RAW_SHAPES.items():
            r = rows // NCORES
            put(blob, name, raw[name][b * r:(b + 1) * r])
        put(blob, "bv", bv16)
        put(blob, "ogc", ogc16)
        b32b = b32.copy()
        b32b[:, B32_TC] = float(tcs[b])
        put(blob, "b32", b32b)
        in_maps.append({"blob16": blob})
    return in_maps


LAST_RESULT = None


def _run(inputs, **kw):
    global LAST_RESULT
    tcs = np.asarray(inputs["true_counts"]).astype(np.int64).reshape(B)
    NT = min(KH, max(1, int(-(-int(tcs.max()) // P))))
    LP = NT * P
    nc = _get_program(NT)
    in_maps = _prep_inputs(inputs, NT)
    res = run_bass_kernel_spmd(nc, in_maps, core_ids=list(range(B)), **kw)
    LAST_RESULT = res
    out = np.zeros((B, L, H), np.float32)
    for b in range(B):
        q = res.results[b]["out"]
        inv = q[H:H + 4].reshape(-1).view(np.float32)
        out[b, :LP] = q[:H].T
        out[b, :LP] *= inv[:, None]
    return out


def kernel(**inputs):
    return _run(inputs)
